# revision 32
# baseline (speedup 1.0000x reference)
"""SmartLinearAppearance Trainium2 kernel (packed ragged-sequence version).

Reference semantics (per (b, n) tracklet, reverse-time scan t = T-1 .. 0):
    xor  = (nv != 0) ^ (v_t != 0)
    prod = nv * v_t
    a_t  = prod * alpha + xor * nv          # per-part coefficient on state
    c_t  = prod * (1 - alpha) + xor * v_t   # per-part coefficient on input
    if m_t: ne = a_t[p] * ne + c_t[p] * e_t ; nv = max(nv, v_t)
    tok = where(any_t m, ne @ W.T + b, 0)

The recurrence is linear in embs given coefficients derived only from
(vis, masks), so it becomes a single weighted reduction:
    ne[n, d] = sum_t w[n, t, p(d)] * embs[n, t, d]
    w = c * cumprod_{t' < t}(m ? a : 1)  (exclusive, ascending t)

Masked-out steps contribute nothing (w = 0), so the host packs each
tracklet's valid timesteps contiguously (ascending t), sorts tracklets
by valid length, and pads per group-of-8 to a length Tg — the embs HBM
read (the roofline term) shrinks to sum(8 * Tg * D) instead of N*T*D.

On-chip, the per-(n,t,p) weights are computed in (p-major, t-minor)
layout on the Vector engine, transposed per-part via the PE (identity
matmul) into PSUM, and assembled into per-group block-diagonal weight
tiles with small copies — no DRAM round trip. Stage 1 contracts each
tracklet pair over (2*Tg) packed steps; stage 2 applies the Linear with
the bias preloaded into PSUM.

Sharding: data-parallel over B across the 8 cores; Linear weights are
replicated (W pre-transposed to bf16 on the host). Outputs are
un-permuted on the host.
"""

import sys

sys.path.insert(0, "/opt/trn_rl_repo")

import functools

import ml_dtypes
import numpy as np

import concourse.bacc as bacc
import concourse.bass as bass
import concourse.tile as tile
from concourse import masks as bass_masks
from concourse import mybir
from concourse.bass_utils import run_bass_kernel_spmd

B, N, T, D, V, TOK = 8, 64, 64, 1792, 7, 512
P = 7          # parts; F = D // P = 256
F = D // P
ALPHA = float(np.float32(0.9))
ONE_MINUS_ALPHA = float(np.float32(1.0) - np.float32(0.9))
NG = 8                   # tracklet groups (8 tracklets each)
GS = N // NG             # group size
DC = D // 128            # 14 d-chunks of 128

f32 = mybir.dt.float32
bf16 = mybir.dt.bfloat16


def _ap(t, offset_elems, dims):
    """Raw AP on a DRAM tensor/tile: dims = [[step, count], ...] in elements."""
    base = t[:] if hasattr(t, "shape") else t
    return bass.AP(tensor=base.tensor, offset=base.offset + offset_elems, ap=dims)


def build_nc(Tp, Tgs):
    nc = bacc.Bacc()

    tot = sum(GS * Tg * D for Tg in Tgs)
    embs_c = nc.dram_tensor("embs_c", [tot], bf16, kind="ExternalInput")
    CW = 5 * Tp + 6
    combo_c = nc.dram_tensor("combo_c", [128, CW], f32, kind="ExternalInput")
    wt_c = nc.dram_tensor("wt_c", [128, DC * TOK], bf16, kind="ExternalInput")

    bb_c = nc.dram_tensor("bb_c", [N, TOK], bf16, kind="ExternalInput")
    out_c = nc.dram_tensor("out_c", [N, TOK], f32, kind="ExternalOutput")

    PADS = 32  # suffix-max doubling scratch pad (max shift)

    with tile.TileContext(nc) as tc:
        with (
            tc.tile_pool(name="small", bufs=1) as small,
            tc.tile_pool(name="big", bufs=1) as bigp,
            tc.tile_pool(name="ps", bufs=1, space="PSUM") as ps,
        ):
            # ---- DMA issues: weights first (needed by mid-kernel stage 2),
            # then the 8 packed embs groups split across scalar + gpsimd so
            # descriptor generation overlaps. sync carries the small inputs.
            combo = small.tile([128, CW], f32)
            nc.scalar.dma_start(out=combo, in_=combo_c[:, :])
            bb_sb = small.tile([N, TOK], bf16)
            nc.scalar.dma_start(out=bb_sb, in_=bb_c[:, :])
            vis = _ap(combo, 0, [combo.ap[0][:], [Tp, 4], [1, Tp]])
            msk = _ap(combo, 4 * Tp, [combo.ap[0][:], [1, Tp]])
            pm = _ap(combo, 5 * Tp, [combo.ap[0][:], [1, 2]])
            pm4 = _ap(combo, 5 * Tp + 2, [combo.ap[0][:], [1, 4]])

            Rs = [4 if Tg <= 32 else 2 for Tg in Tgs]
            wt_sb = bigp.tile([128, DC, TOK], bf16)
            ets = []
            og = 0
            for g in range(NG):
                Tg, R = Tgs[g], Rs[g]
                U = GS // R
                et = bigp.tile([R * Tg, U, D], bf16, name=f"et{g}")
                for u in range(U):
                    nc.gpsimd.dma_start(
                        out=_ap(et, u * D, [et.ap[0][:], [1, D]]),
                        in_=_ap(embs_c, og + u * D,
                                [[Tg * U * D, R], [U * D, Tg], [1, D]]),
                    )
                ets.append(et)
                og += GS * Tg * D
                if g == 3:
                    nc.scalar.dma_start(
                        out=_ap(wt_sb, 0, [wt_sb.ap[0][:], [1, DC * TOK]]),
                        in_=_ap(wt_c, 0, [[DC * TOK, 128], [1, DC * TOK]]),
                    )

            ident = small.tile([128, 128], bf16)
            bass_masks.make_identity(nc, ident[:, :])
            wbds = []
            for g in range(NG):
                Tg, R = Tgs[g], Rs[g]
                wbd = small.tile([R * Tg, GS // R, R, V], bf16, name=f"wbd{g}")
                wbds.append(wbd)

            # mask broadcast view over parts: [N, V(p step 0), Tp]
            mb = bass.AP(tensor=msk.tensor, offset=msk.offset,
                         ap=[msk.ap[0][:], [0, V], [1, Tp]])

            # ---- coefficient chain in (p-major, t-minor) layout on all
            # 128 partitions: parts 0-3 on rows 0-63, parts 4-6 (+pad) on
            # rows 64-127 (host ships vis/mask/pm duplicated accordingly) ----
            VH = 4
            sA = small.tile([128, VH, Tp + PADS], f32)
            sB = small.tile([128, VH, Tp + PADS], f32)
            n0 = small.tile([128, VH, Tp], f32)
            v0 = small.tile([128, VH, Tp], f32)
            xr = small.tile([128, VH, Tp], f32)
            prod = small.tile([128, VH, Tp], f32)
            xnv = small.tile([128, VH, Tp], f32)
            av = small.tile([128, VH, Tp], f32)
            xv = small.tile([128, VH, Tp], f32)
            cc = small.tile([128, VH, Tp], f32)
            gb = small.tile([128, VH, 1 + Tp], f32)
            pb = small.tile([128, VH, Tp], f32)
            mc = small.tile([128, VH, Tp], f32)
            wle = small.tile([128, VH, Tp], f32)
            wle_ms = {}
            for L, R in sorted(set(zip(Tgs, Rs))):
                wle_ms[(L, R)] = small.tile([128, VH, R, L], bf16,
                                            name=f"wle_m{R}_{L}")

            TPS = Tp + PADS

            def sl(t, p0, pn, inner, off=0):
                return _ap(t, p0 * inner + off,
                           [t.ap[0][:], [inner, pn], [1, inner - off]])

            for eng, p0, pn in ((nc.vector, 0, VH),):
                eng.memset(_ap(sA, p0 * TPS, [sA.ap[0][:], [1, pn * TPS]]), 0.0)
                eng.memset(_ap(sB, p0 * TPS, [sB.ap[0][:], [1, pn * TPS]]), 0.0)
                eng.tensor_copy(
                    out=_ap(sA, p0 * TPS, [sA.ap[0][:], [TPS, pn], [1, Tp - 1]]),
                    in_=_ap(vis, p0 * Tp + 1,
                            [vis.ap[0][:], [Tp, pn], [1, Tp - 1]]))
                src, dst = sA, sB
                k = 1
                while k < Tp:
                    eng.tensor_tensor(
                        out=_ap(dst, p0 * TPS, [dst.ap[0][:], [TPS, pn], [1, Tp]]),
                        in0=_ap(src, p0 * TPS, [src.ap[0][:], [TPS, pn], [1, Tp]]),
                        in1=_ap(src, p0 * TPS + k,
                                [src.ap[0][:], [TPS, pn], [1, Tp]]),
                        op=mybir.AluOpType.max)
                    src, dst = dst, src
                    k *= 2
                nv = _ap(src, p0 * TPS, [src.ap[0][:], [TPS, pn], [1, Tp]])
                mbs = _ap(msk, 0, [msk.ap[0][:], [0, pn], [1, Tp]])
                viss = sl(vis, p0, pn, Tp)
                eng.tensor_scalar(out=sl(n0, p0, pn, Tp), in0=nv, scalar1=0.0,
                                  scalar2=None, op0=mybir.AluOpType.is_gt)
                eng.tensor_scalar(out=sl(v0, p0, pn, Tp), in0=viss, scalar1=0.0,
                                  scalar2=None, op0=mybir.AluOpType.is_gt)
                eng.tensor_tensor(out=sl(xr, p0, pn, Tp), in0=sl(n0, p0, pn, Tp),
                                  in1=sl(v0, p0, pn, Tp),
                                  op=mybir.AluOpType.not_equal)
                eng.tensor_tensor(out=sl(prod, p0, pn, Tp), in0=nv, in1=viss,
                                  op=mybir.AluOpType.mult)
                eng.tensor_tensor(out=sl(xnv, p0, pn, Tp), in0=sl(xr, p0, pn, Tp),
                                  in1=nv, op=mybir.AluOpType.mult)
                eng.scalar_tensor_tensor(
                    out=sl(av, p0, pn, Tp), in0=sl(prod, p0, pn, Tp),
                    scalar=ALPHA, in1=sl(xnv, p0, pn, Tp),
                    op0=mybir.AluOpType.mult, op1=mybir.AluOpType.add)
                eng.tensor_tensor(out=sl(xv, p0, pn, Tp), in0=sl(xr, p0, pn, Tp),
                                  in1=viss, op=mybir.AluOpType.mult)
                eng.scalar_tensor_tensor(
                    out=sl(cc, p0, pn, Tp), in0=sl(prod, p0, pn, Tp),
                    scalar=ONE_MINUS_ALPHA, in1=sl(xv, p0, pn, Tp),
                    op0=mybir.AluOpType.mult, op1=mybir.AluOpType.add)
                eng.memset(_ap(gb, p0 * (1 + Tp),
                               [gb.ap[0][:], [1 + Tp, pn], [1, 1]]), 1.0)
                gb3 = _ap(gb, p0 * (1 + Tp) + 1,
                          [gb.ap[0][:], [1 + Tp, pn], [1, Tp]])
                eng.scalar_tensor_tensor(
                    out=gb3, in0=sl(av, p0, pn, Tp), scalar=1.0, in1=mbs,
                    op0=mybir.AluOpType.subtract, op1=mybir.AluOpType.mult)
                eng.tensor_scalar(out=gb3, in0=gb3, scalar1=1.0, scalar2=None,
                                  op0=mybir.AluOpType.add)
                for p in range(p0, p0 + pn):
                    dview = _ap(gb, p * (1 + Tp), [gb.ap[0][:], [1, Tp]])
                    oview = _ap(pb, p * Tp, [pb.ap[0][:], [1, Tp]])
                    eng.tensor_tensor_scan(
                        out=oview, data0=dview, data1=dview, initial=1.0,
                        op0=mybir.AluOpType.mult, op1=mybir.AluOpType.bypass)
                eng.tensor_tensor(out=sl(mc, p0, pn, Tp), in0=sl(cc, p0, pn, Tp),
                                  in1=mbs, op=mybir.AluOpType.mult)
                eng.tensor_tensor(out=sl(wle, p0, pn, Tp),
                                  in0=sl(mc, p0, pn, Tp), in1=sl(pb, p0, pn, Tp),
                                  op=mybir.AluOpType.mult)
                for (L, R), wm in wle_ms.items():
                    pmR = pm4 if R == 4 else pm
                    eng.tensor_tensor(
                        out=_ap(wm, p0 * R * L,
                                [wm.ap[0][:], [R * L, pn], [L, R], [1, L]]),
                        in0=_ap(wle, p0 * Tp,
                                [wle.ap[0][:], [Tp, pn], [0, R], [1, L]]),
                        in1=_ap(pmR, 0, [pmR.ap[0][:], [0, pn], [1, R], [0, L]]),
                        op=mybir.AluOpType.mult)

            # nm = any(mask) per tracklet
            nm = small.tile([N, 1], f32)
            nc.vector.tensor_reduce(out=nm, in_=msk[0:N, :],
                                    axis=mybir.AxisListType.X,
                                    op=mybir.AluOpType.max)

            # bias preloaded into PSUM; stage 2 accumulates on top
            tok_ps = ps.tile([N, TOK], f32)
            nc.vector.tensor_copy(out=tok_ps, in_=bb_sb)
            GV = GS * V
            wbd_ps = ps.tile([128, NG, GV], f32)
            pspitch = wbd_ps[:].ap[0][0]
            for g in range(NG):
                Tg, R = Tgs[g], Rs[g]
                U = GS // R
                wm = wle_ms[(Tg, R)]
                for p in range(V):
                    h, j = divmod(p, VH)
                    nc.tensor.matmul(
                        out=_ap(wbd_ps, g * GV + p,
                                [[pspitch, R * Tg], [R * V, U], [V, R]]),
                        lhsT=_ap(wm, j * R * Tg,
                                 [wm.ap[0][:], [1, R * Tg]]),
                        rhs=ident[:, h * N + GS * g:h * N + GS * (g + 1)],
                        start=True, stop=True)
                nc.vector.tensor_copy(
                    out=wbds[g],
                    in_=_ap(wbd_ps, g * GV, [[pspitch, R * Tg], [1, GV]]))

            # ---- stage 1: neT[d, s] = sum_t w[s, t, p(d)] * embs[s, t, d] ----
            neT_ps = ps.tile([128, DC, N], f32)
            neT_sb = small.tile([128, DC, N], bf16)
            tok_sb = small.tile([N, TOK], f32)
            for g in range(NG):
                et = ets[g]
                wbd = wbds[g]
                R = Rs[g]
                for u in range(GS // R):
                    c0 = GS * g + R * u
                    for dc in range(DC):
                        nc.tensor.matmul(
                            out=neT_ps[:, dc, c0:c0 + R],
                            lhsT=et[:, u, dc * 128:(dc + 1) * 128],
                            rhs=wbd[:, u, :, dc // 2],
                            start=True, stop=True)
                nc.vector.tensor_copy(
                    out=neT_sb[:, :, GS * g:GS * (g + 1)],
                    in_=neT_ps[:, :, GS * g:GS * (g + 1)])
            for kc in range(2):
                ks = slice(kc * (TOK // 2), (kc + 1) * (TOK // 2))
                for dc in range(DC):
                    nc.tensor.matmul(
                        out=tok_ps[:, ks],
                        lhsT=neT_sb[:, dc, :],
                        rhs=wt_sb[:, dc, ks],
                        start=False, stop=(dc == DC - 1),
                        skip_group_check=True)
                nc.vector.tensor_scalar_mul(
                    out=tok_sb[:, ks], in0=tok_ps[:, ks], scalar1=nm)
                nc.sync.dma_start(out=out_c[:, ks], in_=tok_sb[:, ks])


    nc.compile()
    return nc


@functools.lru_cache(maxsize=4)
def _get_nc(Tp, Tgs):
    return build_nc(Tp, Tgs)


def _plan(masks):
    lens = masks.sum(axis=2)                              # [B, N]
    perm = np.argsort(-lens, axis=1, kind="stable")       # [B, N]
    slens = np.take_along_axis(lens, perm, axis=1)
    gmax = slens.reshape(B, NG, GS).max(axis=2).max(axis=0)
    Tgs = np.maximum(np.minimum(((gmax + 7) // 8) * 8, T), 8).astype(int)
    return perm, tuple(int(x) for x in Tgs)


def _prep_in_maps(embs, vis, masks, W, b, perm, Tgs):
    Tp = max(Tgs)
    wt = np.ascontiguousarray(
        W.T.astype(ml_dtypes.bfloat16).reshape(DC, 128, TOK)
        .transpose(1, 0, 2).reshape(128, DC * TOK))
    bb = np.ascontiguousarray(
        np.broadcast_to(b.astype(ml_dtypes.bfloat16), (N, TOK)))
    pmask = np.zeros((N, 2), np.float32)
    pmask[0::2, 0] = 1.0
    pmask[1::2, 1] = 1.0
    pmask = np.tile(pmask, (2, 1))
    pmask4 = np.zeros((N, 4), np.float32)
    for r in range(4):
        pmask4[r::4, r] = 1.0
    pmask4 = np.tile(pmask4, (2, 1))
    tot = sum(GS * Tg * D for Tg in Tgs)
    in_maps = []
    CW = 5 * Tp + 6
    for c in range(B):
        embs_p = np.zeros(tot, ml_dtypes.bfloat16)
        combo = np.zeros((128, CW), np.float32)
        vis_p = combo[:, 0:4 * Tp].reshape(128, 4, Tp)
        mask_p = combo[:, 4 * Tp:5 * Tp]
        combo[:, 5 * Tp:5 * Tp + 2] = pmask
        combo[:, 5 * Tp + 2:5 * Tp + 6] = pmask4
        og = 0
        for g in range(NG):
            Tg = Tgs[g]
            R = 4 if Tg <= 32 else 2
            U = GS // R
            cur = np.zeros((GS, Tg, D), ml_dtypes.bfloat16)
            for j in range(GS):
                s = GS * g + j
                n = perm[c, s]
                ts = np.flatnonzero(masks[c, n])
                l = len(ts)
                cur[j, :l] = embs[c, n, ts].astype(ml_dtypes.bfloat16)
                vt = vis[c, n, ts].T
                vis_p[s, :, :l] = vt[0:4]
                vis_p[N + s, 0:3, :l] = vt[4:7]
                mask_p[s, :l] = 1.0
                mask_p[N + s, :l] = 1.0
            blk = embs_p[og:og + GS * Tg * D].reshape(R, Tg, U, D)
            blk[:] = cur.reshape(U, R, Tg, D).transpose(1, 2, 0, 3)
            og += GS * Tg * D
        in_maps.append({
            "embs_c": embs_p,
            "combo_c": combo,
            "wt_c": wt,
            "bb_c": bb,
        })
    return in_maps


def run(embs, vis, masks, W, b, **run_kwargs):
    perm, Tgs = _plan(masks)
    nc = _get_nc(max(Tgs), Tgs)
    in_maps = _prep_in_maps(embs, vis, masks, W, b, perm, Tgs)
    res = run_bass_kernel_spmd(nc, in_maps, core_ids=list(range(B)),
                               **run_kwargs)
    out = np.empty((B, N, TOK), np.float32)
    for c in range(B):
        out[c][perm[c]] = res.results[c]["out_c"]
    return out, res


def kernel(embs, vis, masks, W, b):
    out, _ = run(embs, vis, masks, W, b)
    return out


# revision 33
# speedup vs baseline: 1.1516x; 1.1516x over previous
"""SmartLinearAppearance Trainium2 kernel (packed ragged-sequence version).

Reference semantics (per (b, n) tracklet, reverse-time scan t = T-1 .. 0):
    xor  = (nv != 0) ^ (v_t != 0)
    prod = nv * v_t
    a_t  = prod * alpha + xor * nv          # per-part coefficient on state
    c_t  = prod * (1 - alpha) + xor * v_t   # per-part coefficient on input
    if m_t: ne = a_t[p] * ne + c_t[p] * e_t ; nv = max(nv, v_t)
    tok = where(any_t m, ne @ W.T + b, 0)

The recurrence is linear in embs given coefficients derived only from
(vis, masks), so it becomes a single weighted reduction:
    ne[n, d] = sum_t w[n, t, p(d)] * embs[n, t, d]
    w = c * cumprod_{t' < t}(m ? a : 1)  (exclusive, ascending t)

Masked-out steps contribute nothing (w = 0), so the host packs each
tracklet's valid timesteps contiguously (ascending t), sorts tracklets
by valid length, and pads per group-of-8 to a length Tg — the embs HBM
read (the roofline term) shrinks to sum(8 * Tg * D) instead of N*T*D.

On-chip, the per-(n,t,p) weights are computed in (p-major, t-minor)
layout on the Vector engine, transposed per-part via the PE (identity
matmul) into PSUM, and assembled into per-group block-diagonal weight
tiles with small copies — no DRAM round trip. Stage 1 contracts each
tracklet pair over (2*Tg) packed steps; stage 2 applies the Linear with
the bias preloaded into PSUM.

Sharding: data-parallel over B across the 8 cores; Linear weights are
replicated (W pre-transposed to bf16 on the host). Outputs are
un-permuted on the host.
"""

import sys

sys.path.insert(0, "/opt/trn_rl_repo")

import functools

import ml_dtypes
import numpy as np

import concourse.bacc as bacc
import concourse.bass as bass
import concourse.tile as tile
from concourse import masks as bass_masks
from concourse import mybir
from concourse.bass_utils import run_bass_kernel_spmd

B, N, T, D, V, TOK = 8, 64, 64, 1792, 7, 512
P = 7          # parts; F = D // P = 256
F = D // P
ALPHA = float(np.float32(0.9))
ONE_MINUS_ALPHA = float(np.float32(1.0) - np.float32(0.9))
NG = 8                   # tracklet groups (8 tracklets each)
GS = N // NG             # group size
DC = D // 128            # 14 d-chunks of 128

f32 = mybir.dt.float32
bf16 = mybir.dt.bfloat16


def _ap(t, offset_elems, dims):
    """Raw AP on a DRAM tensor/tile: dims = [[step, count], ...] in elements."""
    base = t[:] if hasattr(t, "shape") else t
    return bass.AP(tensor=base.tensor, offset=base.offset + offset_elems, ap=dims)


def build_nc(Tp, Tgs):
    nc = bacc.Bacc()

    tot = sum(GS * Tg * D for Tg in Tgs)
    embs_c = nc.dram_tensor("embs_c", [tot], bf16, kind="ExternalInput")
    CW = 5 * Tp + 6
    combo_c = nc.dram_tensor("combo_c", [128, CW], f32, kind="ExternalInput")
    wt_c = nc.dram_tensor("wt_c", [128, DC * TOK], bf16, kind="ExternalInput")

    bb_c = nc.dram_tensor("bb_c", [N, TOK], bf16, kind="ExternalInput")
    out_c = nc.dram_tensor("out_c", [N, TOK], f32, kind="ExternalOutput")

    PADS = 32  # suffix-max doubling scratch pad (max shift)

    with tile.TileContext(nc) as tc:
        with (
            tc.tile_pool(name="small", bufs=1) as small,
            tc.tile_pool(name="big", bufs=1) as bigp,
            tc.tile_pool(name="ps", bufs=1, space="PSUM") as ps,
        ):
            # ---- DMA issues: weights first (needed by mid-kernel stage 2),
            # then the 8 packed embs groups split across scalar + gpsimd so
            # descriptor generation overlaps. sync carries the small inputs.
            combo = small.tile([128, CW], f32)
            nc.scalar.dma_start(out=combo, in_=combo_c[:, :])
            bb_sb = small.tile([N, TOK], bf16)
            nc.scalar.dma_start(out=bb_sb, in_=bb_c[:, :])
            vis = _ap(combo, 0, [combo.ap[0][:], [Tp, 4], [1, Tp]])
            msk = _ap(combo, 4 * Tp, [combo.ap[0][:], [1, Tp]])
            pm = _ap(combo, 5 * Tp, [combo.ap[0][:], [1, 2]])
            pm4 = _ap(combo, 5 * Tp + 2, [combo.ap[0][:], [1, 4]])

            Rs = [4 if Tg <= 32 else 2 for Tg in Tgs]
            wt_sb = bigp.tile([128, DC, TOK], bf16)
            ets = []
            og = 0
            for g in range(NG):
                Tg, R = Tgs[g], Rs[g]
                U = GS // R
                et = bigp.tile([R * Tg, U, D], bf16, name=f"et{g}")
                for u in range(U):
                    nc.scalar.dma_start(
                        out=_ap(et, u * D, [et.ap[0][:], [1, D]]),
                        in_=_ap(embs_c, og + u * D,
                                [[Tg * U * D, R], [U * D, Tg], [1, D]]),
                    )
                ets.append(et)
                og += GS * Tg * D
                if g == 3:
                    nc.scalar.dma_start(
                        out=_ap(wt_sb, 0, [wt_sb.ap[0][:], [1, DC * TOK]]),
                        in_=_ap(wt_c, 0, [[DC * TOK, 128], [1, DC * TOK]]),
                    )

            ident = small.tile([128, 128], bf16)
            bass_masks.make_identity(nc, ident[:, :])
            wbds = []
            for g in range(NG):
                Tg, R = Tgs[g], Rs[g]
                wbd = small.tile([R * Tg, GS // R, R, V], bf16, name=f"wbd{g}")
                wbds.append(wbd)

            # mask broadcast view over parts: [N, V(p step 0), Tp]
            mb = bass.AP(tensor=msk.tensor, offset=msk.offset,
                         ap=[msk.ap[0][:], [0, V], [1, Tp]])

            # ---- coefficient chain in (p-major, t-minor) layout on all
            # 128 partitions: parts 0-3 on rows 0-63, parts 4-6 (+pad) on
            # rows 64-127 (host ships vis/mask/pm duplicated accordingly) ----
            VH = 4
            sA = small.tile([128, VH, Tp + PADS], f32)
            sB = small.tile([128, VH, Tp + PADS], f32)
            n0 = small.tile([128, VH, Tp], f32)
            v0 = small.tile([128, VH, Tp], f32)
            xr = small.tile([128, VH, Tp], f32)
            prod = small.tile([128, VH, Tp], f32)
            xnv = small.tile([128, VH, Tp], f32)
            av = small.tile([128, VH, Tp], f32)
            xv = small.tile([128, VH, Tp], f32)
            cc = small.tile([128, VH, Tp], f32)
            gb = small.tile([128, VH, 1 + Tp], f32)
            pb = small.tile([128, VH, Tp], f32)
            mc = small.tile([128, VH, Tp], f32)
            wle = small.tile([128, VH, Tp], f32)
            wle_ms = {}
            for L, R in sorted(set(zip(Tgs, Rs))):
                wle_ms[(L, R)] = small.tile([128, VH, R, L], bf16,
                                            name=f"wle_m{R}_{L}")

            TPS = Tp + PADS

            def sl(t, p0, pn, inner, off=0):
                return _ap(t, p0 * inner + off,
                           [t.ap[0][:], [inner, pn], [1, inner - off]])

            for eng, p0, pn in ((nc.vector, 0, VH),):
                eng.memset(_ap(sA, p0 * TPS, [sA.ap[0][:], [1, pn * TPS]]), 0.0)
                eng.memset(_ap(sB, p0 * TPS, [sB.ap[0][:], [1, pn * TPS]]), 0.0)
                eng.tensor_copy(
                    out=_ap(sA, p0 * TPS, [sA.ap[0][:], [TPS, pn], [1, Tp - 1]]),
                    in_=_ap(vis, p0 * Tp + 1,
                            [vis.ap[0][:], [Tp, pn], [1, Tp - 1]]))
                src, dst = sA, sB
                k = 1
                while k < Tp:
                    eng.tensor_tensor(
                        out=_ap(dst, p0 * TPS, [dst.ap[0][:], [TPS, pn], [1, Tp]]),
                        in0=_ap(src, p0 * TPS, [src.ap[0][:], [TPS, pn], [1, Tp]]),
                        in1=_ap(src, p0 * TPS + k,
                                [src.ap[0][:], [TPS, pn], [1, Tp]]),
                        op=mybir.AluOpType.max)
                    src, dst = dst, src
                    k *= 2
                nv = _ap(src, p0 * TPS, [src.ap[0][:], [TPS, pn], [1, Tp]])
                mbs = _ap(msk, 0, [msk.ap[0][:], [0, pn], [1, Tp]])
                viss = sl(vis, p0, pn, Tp)
                eng.tensor_scalar(out=sl(n0, p0, pn, Tp), in0=nv, scalar1=0.0,
                                  scalar2=None, op0=mybir.AluOpType.is_gt)
                eng.tensor_scalar(out=sl(v0, p0, pn, Tp), in0=viss, scalar1=0.0,
                                  scalar2=None, op0=mybir.AluOpType.is_gt)
                eng.tensor_tensor(out=sl(xr, p0, pn, Tp), in0=sl(n0, p0, pn, Tp),
                                  in1=sl(v0, p0, pn, Tp),
                                  op=mybir.AluOpType.not_equal)
                eng.tensor_tensor(out=sl(prod, p0, pn, Tp), in0=nv, in1=viss,
                                  op=mybir.AluOpType.mult)
                eng.tensor_tensor(out=sl(xnv, p0, pn, Tp), in0=sl(xr, p0, pn, Tp),
                                  in1=nv, op=mybir.AluOpType.mult)
                eng.scalar_tensor_tensor(
                    out=sl(av, p0, pn, Tp), in0=sl(prod, p0, pn, Tp),
                    scalar=ALPHA, in1=sl(xnv, p0, pn, Tp),
                    op0=mybir.AluOpType.mult, op1=mybir.AluOpType.add)
                eng.tensor_tensor(out=sl(xv, p0, pn, Tp), in0=sl(xr, p0, pn, Tp),
                                  in1=viss, op=mybir.AluOpType.mult)
                eng.scalar_tensor_tensor(
                    out=sl(cc, p0, pn, Tp), in0=sl(prod, p0, pn, Tp),
                    scalar=ONE_MINUS_ALPHA, in1=sl(xv, p0, pn, Tp),
                    op0=mybir.AluOpType.mult, op1=mybir.AluOpType.add)
                eng.memset(_ap(gb, p0 * (1 + Tp),
                               [gb.ap[0][:], [1 + Tp, pn], [1, 1]]), 1.0)
                gb3 = _ap(gb, p0 * (1 + Tp) + 1,
                          [gb.ap[0][:], [1 + Tp, pn], [1, Tp]])
                eng.scalar_tensor_tensor(
                    out=gb3, in0=sl(av, p0, pn, Tp), scalar=1.0, in1=mbs,
                    op0=mybir.AluOpType.subtract, op1=mybir.AluOpType.mult)
                eng.tensor_scalar(out=gb3, in0=gb3, scalar1=1.0, scalar2=None,
                                  op0=mybir.AluOpType.add)
                for p in range(p0, p0 + pn):
                    dview = _ap(gb, p * (1 + Tp), [gb.ap[0][:], [1, Tp]])
                    oview = _ap(pb, p * Tp, [pb.ap[0][:], [1, Tp]])
                    eng.tensor_tensor_scan(
                        out=oview, data0=dview, data1=dview, initial=1.0,
                        op0=mybir.AluOpType.mult, op1=mybir.AluOpType.bypass)
                eng.tensor_tensor(out=sl(mc, p0, pn, Tp), in0=sl(cc, p0, pn, Tp),
                                  in1=mbs, op=mybir.AluOpType.mult)
                eng.tensor_tensor(out=sl(wle, p0, pn, Tp),
                                  in0=sl(mc, p0, pn, Tp), in1=sl(pb, p0, pn, Tp),
                                  op=mybir.AluOpType.mult)
                for (L, R), wm in wle_ms.items():
                    pmR = pm4 if R == 4 else pm
                    eng.tensor_tensor(
                        out=_ap(wm, p0 * R * L,
                                [wm.ap[0][:], [R * L, pn], [L, R], [1, L]]),
                        in0=_ap(wle, p0 * Tp,
                                [wle.ap[0][:], [Tp, pn], [0, R], [1, L]]),
                        in1=_ap(pmR, 0, [pmR.ap[0][:], [0, pn], [1, R], [0, L]]),
                        op=mybir.AluOpType.mult)

            # nm = any(mask) per tracklet
            nm = small.tile([N, 1], f32)
            nc.vector.tensor_reduce(out=nm, in_=msk[0:N, :],
                                    axis=mybir.AxisListType.X,
                                    op=mybir.AluOpType.max)

            # bias preloaded into PSUM; stage 2 accumulates on top
            tok_ps = ps.tile([N, TOK], f32)
            nc.vector.tensor_copy(out=tok_ps, in_=bb_sb)
            GV = GS * V
            wbd_ps = ps.tile([128, NG, GV], f32)
            pspitch = wbd_ps[:].ap[0][0]
            for g in range(NG):
                Tg, R = Tgs[g], Rs[g]
                U = GS // R
                wm = wle_ms[(Tg, R)]
                for p in range(V):
                    h, j = divmod(p, VH)
                    nc.tensor.matmul(
                        out=_ap(wbd_ps, g * GV + p,
                                [[pspitch, R * Tg], [R * V, U], [V, R]]),
                        lhsT=_ap(wm, j * R * Tg,
                                 [wm.ap[0][:], [1, R * Tg]]),
                        rhs=ident[:, h * N + GS * g:h * N + GS * (g + 1)],
                        start=True, stop=True)
                nc.vector.tensor_copy(
                    out=wbds[g],
                    in_=_ap(wbd_ps, g * GV, [[pspitch, R * Tg], [1, GV]]))

            # ---- stage 1: neT[d, s] = sum_t w[s, t, p(d)] * embs[s, t, d] ----
            neT_ps = ps.tile([128, DC, N], f32)
            neT_sb = small.tile([128, DC, N], bf16)
            tok_sb = small.tile([N, TOK], f32)
            for g in range(NG):
                et = ets[g]
                wbd = wbds[g]
                R = Rs[g]
                for u in range(GS // R):
                    c0 = GS * g + R * u
                    for dc in range(DC):
                        nc.tensor.matmul(
                            out=neT_ps[:, dc, c0:c0 + R],
                            lhsT=et[:, u, dc * 128:(dc + 1) * 128],
                            rhs=wbd[:, u, :, dc // 2],
                            start=True, stop=True)
                nc.vector.tensor_copy(
                    out=neT_sb[:, :, GS * g:GS * (g + 1)],
                    in_=neT_ps[:, :, GS * g:GS * (g + 1)])
            for kc in range(2):
                ks = slice(kc * (TOK // 2), (kc + 1) * (TOK // 2))
                for dc in range(DC):
                    nc.tensor.matmul(
                        out=tok_ps[:, ks],
                        lhsT=neT_sb[:, dc, :],
                        rhs=wt_sb[:, dc, ks],
                        start=False, stop=(dc == DC - 1),
                        skip_group_check=True)
                nc.vector.tensor_scalar_mul(
                    out=tok_sb[:, ks], in0=tok_ps[:, ks], scalar1=nm)
                nc.sync.dma_start(out=out_c[:, ks], in_=tok_sb[:, ks])


    nc.compile()
    return nc


@functools.lru_cache(maxsize=4)
def _get_nc(Tp, Tgs):
    return build_nc(Tp, Tgs)


def _plan(masks):
    lens = masks.sum(axis=2)                              # [B, N]
    perm = np.argsort(-lens, axis=1, kind="stable")       # [B, N]
    slens = np.take_along_axis(lens, perm, axis=1)
    gmax = slens.reshape(B, NG, GS).max(axis=2).max(axis=0)
    Tgs = np.maximum(np.minimum(((gmax + 7) // 8) * 8, T), 8).astype(int)
    return perm, tuple(int(x) for x in Tgs)


def _prep_in_maps(embs, vis, masks, W, b, perm, Tgs):
    Tp = max(Tgs)
    wt = np.ascontiguousarray(
        W.T.astype(ml_dtypes.bfloat16).reshape(DC, 128, TOK)
        .transpose(1, 0, 2).reshape(128, DC * TOK))
    bb = np.ascontiguousarray(
        np.broadcast_to(b.astype(ml_dtypes.bfloat16), (N, TOK)))
    pmask = np.zeros((N, 2), np.float32)
    pmask[0::2, 0] = 1.0
    pmask[1::2, 1] = 1.0
    pmask = np.tile(pmask, (2, 1))
    pmask4 = np.zeros((N, 4), np.float32)
    for r in range(4):
        pmask4[r::4, r] = 1.0
    pmask4 = np.tile(pmask4, (2, 1))
    tot = sum(GS * Tg * D for Tg in Tgs)
    in_maps = []
    CW = 5 * Tp + 6
    for c in range(B):
        embs_p = np.zeros(tot, ml_dtypes.bfloat16)
        combo = np.zeros((128, CW), np.float32)
        vis_p = combo[:, 0:4 * Tp].reshape(128, 4, Tp)
        mask_p = combo[:, 4 * Tp:5 * Tp]
        combo[:, 5 * Tp:5 * Tp + 2] = pmask
        combo[:, 5 * Tp + 2:5 * Tp + 6] = pmask4
        og = 0
        for g in range(NG):
            Tg = Tgs[g]
            R = 4 if Tg <= 32 else 2
            U = GS // R
            cur = np.zeros((GS, Tg, D), ml_dtypes.bfloat16)
            for j in range(GS):
                s = GS * g + j
                n = perm[c, s]
                ts = np.flatnonzero(masks[c, n])
                l = len(ts)
                cur[j, :l] = embs[c, n, ts].astype(ml_dtypes.bfloat16)
                vt = vis[c, n, ts].T
                vis_p[s, :, :l] = vt[0:4]
                vis_p[N + s, 0:3, :l] = vt[4:7]
                mask_p[s, :l] = 1.0
                mask_p[N + s, :l] = 1.0
            blk = embs_p[og:og + GS * Tg * D].reshape(R, Tg, U, D)
            blk[:] = cur.reshape(U, R, Tg, D).transpose(1, 2, 0, 3)
            og += GS * Tg * D
        in_maps.append({
            "embs_c": embs_p,
            "combo_c": combo,
            "wt_c": wt,
            "bb_c": bb,
        })
    return in_maps


def run(embs, vis, masks, W, b, **run_kwargs):
    perm, Tgs = _plan(masks)
    nc = _get_nc(max(Tgs), Tgs)
    in_maps = _prep_in_maps(embs, vis, masks, W, b, perm, Tgs)
    res = run_bass_kernel_spmd(nc, in_maps, core_ids=list(range(B)),
                               **run_kwargs)
    out = np.empty((B, N, TOK), np.float32)
    for c in range(B):
        out[c][perm[c]] = res.results[c]["out_c"]
    return out, res


def kernel(embs, vis, masks, W, b):
    out, _ = run(embs, vis, masks, W, b)
    return out
